# revision 2
# baseline (speedup 1.0000x reference)
"""Trainium2 Bass kernel for nn_LongformerPersonalizedClsHead (MoE routing head).

Reference computation (B=256, S=512, H=768, U=100, L=2):
    x  = hidden_states[:, 0, :]                      # [B, H]  (CLS token only)
    z  = sum_u mask[b,u] * (x @ dense_W[u]) + mask @ dense_b
    h  = tanh(z)
    out= sum_u mask[b,u] * (h @ out_proj_W[u]) + mask @ out_proj_b   # [B, L]

Sharding: each core owns 96 k-columns of layer 1 (dense) output; the expert
sum is local per core, and layer 2 contracts over k so each core emits an
independent [256, 2+2] partial that the host sums. Zero on-device collectives.

fp8 hi/lo residual scheme: W and x are split on the host into
fp8e4m3 hi + fp8e4m3 lo (lo = quantization residual stored at the SAME scale,
so all passes share one PSUM accumulation group). Per (expert, batch-tile) the
PE runs 9 DoubleRow matmuls (contraction 256 each, 20 ns apiece at full
p-state) covering x_hi*w_hi + x_hi*w_lo + x_lo*w_hi; the dropped lo*lo term
is ~0.1% relative. End-to-end rel err ~3e-3 (bf16 baseline was 4.3e-3) while
PE drops from 48us (bf16) to 36us, under the ~42.6us DMA stream.

Combine chains: mask-weighted accumulation of each expert's PSUM tile into z
runs on ACT(mul)+Pool(add) for batch-tile 0 and fused DVE stt for tile 1,
with TWO ping-pong accumulators per tile (a serial z+= chain self-paces at
~484ns/step, slower than the ~426ns/expert DMA pace; alternating experts
between two accumulators halves each chain's step rate).

Tail: Wo_comb[b,(l,k)] = sum_u mask[b,u]*out_proj_W[u,k,l] and the output
bias are precomputed early on the PE (one 194-wide matmul per batch tile from
maskT/woT that arrive in the DMA head), so the tail is just
tanh -> 2 fused multiply-reduce ops per tile -> one small writeback. No
transpose, no layer-2 matmul, no hT copy on the critical path.
"""
import numpy as np

B, S, H, U, L = 256, 512, 768, 100, 2
N_CORES = 8
KSL = H // N_CORES    # 96 k-columns per core
NB = B // 128         # 2 batch tiles
NCP = H // 256        # 3 contraction chunk-pairs (DoubleRow contracts 256)
WOC = 2 * KSL         # 192 = (l, k) columns of the combined layer-2 weight

# DMA chunk sizes for the expert-weight stream (experts per DMA). 2-expert
# chunks (819ns transfer) keep HWDGE (625ns/DMA) off the critical path. The
# tail shrinks GRADUALLY (PE time of chunk n, 360ns/expert, must stay under
# the transfer time of chunk n+1, 410ns/expert, or PE piles up behind the
# per-chunk +900ns DMA-sem latency) so the final expert's matmuls start
# 900ns after the last arrival with no backlog.
NSINGLE = 18          # experts stored hi-only (no w_lo pass): rel err ~1.4e-2,
                      # saves 205ns of stream per expert
NDUAL = U - NSINGLE
# single-expert chunks (hi-only, 205ns/expert transfer)
WS_CHUNKS = [6, 6, 6]
assert sum(WS_CHUNKS) == NSINGLE
# dual-expert chunks; the taper keeps PE(chunk n) under transfer(chunk n+1)
W_CHUNKS = [2] + [6] * 7 + [5, 5, 4, 4, 3, 3, 3, 2, 2, 2, 2, 2, 1]
assert sum(W_CHUNKS) == NDUAL
TAILK = 2             # last experts run t0-block then t1-block so the t0
                      # merge/tanh (Pool/ACT) overlap the t1 block
TRIGGER_OUT = False   # SWDGE prepare/trigger writeback; broken on real HW (see notes)
OPAD = 64             # o rows padded to 256B for the SWDGE scatter-add writeback

PE_WARMUP = 45        # dummy matmuls to ramp the PE clock during the DMA head
SX, SW = 16.0, 512.0  # fp8 pre-scales for x and W (power of 2, exact)

_RUNNER = None


def _build_nc():
    import concourse.bacc as bacc
    import concourse.mybir as mybir
    import concourse.tile as tile
    from concourse.masks import make_identity

    f32 = mybir.dt.float32
    bf16 = mybir.dt.bfloat16
    fp8 = mybir.dt.float8e4
    DR = mybir.MatmulPerfMode.DoubleRow
    mult = mybir.AluOpType.mult
    add = mybir.AluOpType.add

    nc = bacc.Bacc("TRN2", target_bir_lowering=False)

    # Host-prepacked layouts (p = partition = h % 128 within chunk-pair):
    # x8[p, t, hl, cp, i, b]: h = cp*256 + i*128 + p, b_global = t*128 + b
    x8 = nc.dram_tensor("x8", [128, NB, 2, NCP, 2, 128], fp8, kind="ExternalInput")
    # singles (experts 0..NSINGLE-1), hi only: w8s[p, u, cp, i, k]
    w8s = nc.dram_tensor("w8s", [128, NSINGLE, NCP, 2, KSL], fp8,
                         kind="ExternalInput")
    # duals (experts NSINGLE..U-1): w8[p, u, hl, cp, i, k], hl 0=hi 1=lo
    w8 = nc.dram_tensor("w8", [128, NDUAL, 2, NCP, 2, KSL], fp8,
                        kind="ExternalInput")
    # [maskT(256) | dense_b slice(96) | woT (l,k)(192) | out_proj_b.T/8 (2)]
    md = nc.dram_tensor("md", [U, B + KSL + WOC + L], bf16, kind="ExternalInput")
    # output (rows padded to 256B for the scatter-add writeback):
    # cols [t*L + l] = sum_k h*Wo_comb partials, cols [4 + t*L + l]
    # = (mask @ out_proj_b)/8 bias seeds; host sums cols 0..7.
    o = nc.dram_tensor("o", [128, OPAD], f32, kind="ExternalOutput")

    with tile.TileContext(nc) as tc:
        with (
            tc.tile_pool(name="const", bufs=1) as cpool,
            tc.tile_pool(name="tmp", bufs=6) as tpool,
            tc.tile_pool(name="py", bufs=7, space="PSUM") as psum_y,
            tc.tile_pool(name="pq", bufs=1, space="PSUM") as psum_q,
        ):
            # ones is the first DVE instruction of the program so the PE
            # warmup below can begin as early as possible (the 3us clock ramp
            # must complete before the first expert matmul to pay off)
            ones_first = cpool.tile([1, 128], bf16, tag="ones1")
            nc.vector.memset(ones_first[:], 1.0)

            # --- input DMAs ---
            # SP/HWDGE queue carries the critical stream (x8 halves + expert
            # weights); small inputs ride the Pool/SWDGE queue right behind
            # the first weight chunk.
            x8_sb = cpool.tile([128, NB, 2, NCP, 2, 128], fp8, tag="x8")
            nc.sync.dma_start(x8_sb[:, 0], x8[:, 0])
            ws_sb = cpool.tile([128, NSINGLE, NCP, 2, KSL], fp8, tag="w8s")
            nc.sync.dma_start(ws_sb[:, 0:WS_CHUNKS[0]], w8s[:, 0:WS_CHUNKS[0]])
            nc.sync.dma_start(x8_sb[:, 1], x8[:, 1])
            u0 = WS_CHUNKS[0]
            for n in WS_CHUNKS[1:]:
                nc.sync.dma_start(ws_sb[:, u0:u0 + n], w8s[:, u0:u0 + n])
                u0 += n
            w8_sb = cpool.tile([128, NDUAL, 2, NCP, 2, KSL], fp8, tag="w8")
            u0 = 0
            for n in W_CHUNKS:
                nc.sync.dma_start(w8_sb[:, u0:u0 + n], w8[:, u0:u0 + n])
                u0 += n

            md_sb = cpool.tile([U, B + KSL + WOC + L], bf16, tag="md")
            nc.gpsimd.dma_start(md_sb[:], md[:])
            # chain-combine scalars mask/(SX*SW) are reconstructed on-chip
            # from md's bf16 maskT (PE transpose + ACT scale-copy) instead of
            # paying a separate 284ns DMA in the critical stream
            mask_sb = cpool.tile([128, NB, U], f32, tag="mask")
            maskT_sb = md_sb[:, 0:B]
            db_sb = md_sb[:, B:B + KSL]
            wob_sb = md_sb[:, B + KSL:B + KSL + WOC + L]   # [woT | bo/8]

            o_pad = cpool.tile([128, OPAD], f32, tag="opad")
            nc.vector.memset(o_pad[:], 0.0)

            # SWDGE scatter-add writeback: descriptors prepared up-front on
            # the (idle-early) Pool engine; the end-of-kernel trigger then
            # pays only transfer + semaphore instead of HWDGE + DGE ~1.3us.
            # Token i reads its index from [p=i%16, s=i//16]; the interp views
            # the AP as [128, 8] and bounds-checks every value, so zero the
            # unread partitions >= 16.
            if TRIGGER_OUT:
                # scatter-add needs a known-zero destination; the output DRAM
                # buffer is not guaranteed zero on hardware. The zero-fill DMA
                # rides the SP queue after the last weight chunk (o_pad is
                # still all-zero here; its stt writers get a WAR edge on this
                # read, satisfied ~35us before they run).
                nc.sync.dma_start(o[:], o_pad[:])
                oidx = cpool.tile([128, 128 // 16], mybir.dt.int16, tag="oidx")
                nc.gpsimd.memset(oidx[:], 0)
                nc.gpsimd.iota(oidx[0:16, :], pattern=[[16, 128 // 16]], base=0,
                               channel_multiplier=1)
                dma_sem = nc.alloc_semaphore("swdge_out")
                nc.gpsimd.dma_scatter_add(
                    o[:], o_pad[:].rearrange("p (a e) -> p a e", a=1), oidx[:],
                    128, 128, OPAD, prepare_only=True, sem=dma_sem,
                )

            # z accumulators: [t][pp] with pp the ping-pong slot. Slot 0 is
            # seeded with the dense bias (PE matmul below); slot 1 starts 0.
            z_sb = cpool.tile([128, NB, 2, KSL], f32, tag="z")
            for t in range(NB):
                nc.vector.memset(z_sb[:, t, 1, :], 0.0)
            h_sb = cpool.tile([128, NB, KSL], bf16, tag="h")
            woc_sb = cpool.tile([128, NB, WOC], bf16, tag="woc")

            # p-state warmup: keep PE continuously busy through its ~3us clock
            # ramp while the head DMAs stream
            for _ in range(PE_WARMUP):
                wacc = psum_y.tile([128, KSL], f32, tag="y")
                nc.tensor.matmul(wacc[:], ones_first[:], ones_first[:, 0:KSL],
                                 start=True, stop=True)

            def emit_expert(u, t, dve_combine=False):
                acc = psum_y.tile([128, KSL], f32, tag="y")
                if u < NSINGLE:
                    # hi-only expert: z_u = (x_hi + x_lo) * w_hi
                    steps = [(0, cp) for cp in range(NCP)] + \
                            [(1, cp) for cp in range(NCP)]
                    for n, (xhl, cp) in enumerate(steps):
                        nc.tensor.matmul(
                            acc[:], x8_sb[:, t, xhl, cp], ws_sb[:, u, cp],
                            start=(n == 0), stop=(n == len(steps) - 1),
                            perf_mode=DR,
                        )
                else:
                    steps = [(0, 0, cp) for cp in range(NCP)] + \
                            [(0, 1, cp) for cp in range(NCP)] + \
                            [(1, 0, cp) for cp in range(NCP)]
                    for n, (xhl, whl, cp) in enumerate(steps):
                        nc.tensor.matmul(
                            acc[:],
                            x8_sb[:, t, xhl, cp],
                            w8_sb[:, u - NSINGLE, whl, cp],
                            start=(n == 0), stop=(n == len(steps) - 1),
                            perf_mode=DR,
                        )
                pp = (u + 1) % 2
                if t == 0 and not dve_combine:
                    tmp = tpool.tile([128, KSL], f32, tag="tmp")
                    nc.scalar.mul(tmp[:], acc[:], mask_sb[:, 0, u:u + 1])
                    nc.gpsimd.tensor_add(z_sb[:, 0, pp, :], z_sb[:, 0, pp, :],
                                         tmp[:])
                else:
                    nc.vector.scalar_tensor_tensor(
                        z_sb[:, t, pp, :], acc[:], mask_sb[:, t, u:u + 1],
                        z_sb[:, t, pp, :], op0=mult, op1=add,
                    )

            # --- seeds (PE): mask transpose, z bias, Wo_comb + out bias ---
            # These stall PE ~0.5us on the SWDGE-delivered md tensor, which is
            # free (PE has ~5us of slack against the weight stream). The mask
            # copies MUST precede the first combine mul in the ACT queue.
            ident = cpool.tile([128, 128], bf16, tag="ident")
            make_identity(nc, ident[:])
            for t in range(NB):
                mt = psum_y.tile([128, U], bf16, tag="y")
                nc.tensor.transpose(mt[:], maskT_sb[:, t * 128:(t + 1) * 128],
                                    ident[0:U, 0:U])
                nc.scalar.mul(mask_sb[:, t, :], mt[:], 1.0 / (SX * SW))
            for t in range(NB):
                ps = psum_y.tile([128, KSL], f32, tag="y")
                nc.tensor.matmul(
                    ps[:], maskT_sb[:, t * 128:(t + 1) * 128], db_sb[:],
                    start=True, stop=True,
                )
                nc.scalar.copy(z_sb[:, t, 0, :], ps[:])
            for t in range(NB):
                qp = psum_q.tile([128, WOC + L], f32, tag="q")
                nc.tensor.matmul(
                    qp[:], maskT_sb[:, t * 128:(t + 1) * 128], wob_sb[:],
                    start=True, stop=True,
                )
                nc.vector.tensor_copy(woc_sb[:, t, :], qp[:, 0:WOC])
                nc.scalar.copy(o_pad[:, 4 + t * L:4 + t * L + L],
                               qp[:, WOC:WOC + L])

            emit_expert(0, 0)
            emit_expert(0, 1)
            emit_expert(1, 0)
            emit_expert(1, 1)
            for u in range(2, U - TAILK):
                emit_expert(u, 0)
                emit_expert(u, 1)

            # --- tail ---
            # Last TAILK experts run all-t0 then all-t1; the t0 combines jump
            # to the 1-op DVE stt path so the t0 chain drains without the
            # ACT->Pool 2-hop latency. merge0 on Pool, merge1 on DVE, the two
            # tanh on ACT, and all four multiply-reduce ops on DVE (walrus
            # rejects TensorScalarPtr on Pool).
            def stt_reduce(t, l):
                p2 = tpool.tile([128, KSL], f32, tag="p2")
                nc.vector.scalar_tensor_tensor(
                    p2[:], h_sb[:, t, :], 1.0,
                    woc_sb[:, t, l * KSL:(l + 1) * KSL],
                    op0=mult, op1=mult,
                    accum_out=o_pad[:, t * L + l:t * L + l + 1],
                )

            for u in range(U - TAILK, U):
                emit_expert(u, 0, dve_combine=True)
            nc.gpsimd.tensor_add(z_sb[:, 0, 0, :], z_sb[:, 0, 0, :],
                                 z_sb[:, 0, 1, :])
            nc.scalar.activation(h_sb[:, 0, :], z_sb[:, 0, 0, :],
                                 mybir.ActivationFunctionType.Tanh)
            for u in range(U - TAILK, U):
                emit_expert(u, 1)
            nc.vector.tensor_add(z_sb[:, 1, 0, :], z_sb[:, 1, 0, :],
                                 z_sb[:, 1, 1, :])
            nc.scalar.activation(h_sb[:, 1, :], z_sb[:, 1, 0, :],
                                 mybir.ActivationFunctionType.Tanh)
            for l in range(L):
                stt_reduce(0, l)
            for l in range(L):
                stt_reduce(1, l)
            if TRIGGER_OUT:
                nc.gpsimd.trigger_dma(count=None)
            else:
                nc.sync.dma_start(o[:, 0:8], o_pad[:, 0:8])

    # Tile schedules data consumers of the prepared scatter-add against the
    # prep's DMASW lane sem, but the completion increment it bakes stays on
    # the user-provided sem (on_update[0]), which the trigger-drain fires in
    # both CoreSim and TimelineSim. Rewire every wait on the orphaned DMASW
    # sem to wait on the user sem instead; the DMASW sem then stays untouched
    # (cleared at zero) and the teardown's sync check is satisfied.
    fn = nc.m.functions[0]
    if not TRIGGER_OUT:
        nc.finalize()
        return nc
    upd_ids = set()
    user_sem = None
    for blk in fn.blocks:
        for inst in blk.instructions:
            si = inst.sync_info
            if not si:
                continue
            if "ScatterAdd" in type(inst).__name__:
                user_sem = si.on_update[0]
                assert user_sem.ant_name == "swdge_out", user_sem
            for upd in si.on_update:
                if (upd.ant_name or "").startswith("DMASW"):
                    upd_ids.add(upd.id)
    assert user_sem is not None
    for blk in fn.blocks:
        for inst in blk.instructions:
            si = inst.sync_info
            if not si:
                continue
            for wt in si.on_wait:
                if (wt.ant_name or "").startswith("DMASW") and wt.id not in upd_ids:
                    wt.id, wt.ant_name = user_sem.id, user_sem.ant_name

    nc.finalize()
    return nc


class _SpmdRunner:
    """Cached PJRT SPMD runner (mirrors concourse.bass2jax.run_bass_via_pjrt,
    but keeps the jitted callable alive so repeat calls don't re-trace)."""

    def __init__(self, nc, n_cores):
        import jax
        import concourse.mybir as mybir
        from concourse.bass2jax import (
            _bass_exec_p, install_neuronx_cc_hook, partition_id_tensor,
        )
        from jax.sharding import Mesh, PartitionSpec, NamedSharding
        try:
            from jax.experimental.shard_map import shard_map
        except ImportError:
            from jax.shard_map import shard_map

        install_neuronx_cc_hook()
        self.jax = jax
        self.nc = nc
        self.n_cores = n_cores

        in_names, out_names, out_avals, zero_outs = [], [], [], []
        partition_name = nc.partition_id_tensor.name if nc.partition_id_tensor else None
        dbg_name = None
        if nc.dbg_addr is not None:
            assert not nc.dbg_callbacks
            dbg_name = nc.dbg_addr.name
        for alloc in nc.m.functions[0].allocations:
            if not isinstance(alloc, mybir.MemoryLocationSet):
                continue
            name = alloc.memorylocations[0].name
            if alloc.kind == "ExternalInput":
                if name not in (partition_name, dbg_name):
                    in_names.append(name)
            elif alloc.kind == "ExternalOutput":
                out_names.append(name)
                shape = tuple(alloc.tensor_shape)
                dtype = mybir.dt.np(alloc.dtype)
                out_avals.append(jax.core.ShapedArray(shape, dtype))
                zero_outs.append(np.zeros(shape, dtype))

        self.in_names = list(in_names)
        self.out_names = list(out_names)
        self.zero_outs = zero_outs

        n_params = len(in_names)
        bound_names = list(in_names) + list(out_names)
        if dbg_name is not None:
            bound_names.append(dbg_name)
        if partition_name is not None:
            bound_names.append(partition_name)

        def _body(*args):
            operands = list(args)
            if dbg_name is not None:
                operands.append(jax.numpy.zeros((1, 2), jax.numpy.uint32))
            if partition_name is not None:
                operands.append(partition_id_tensor())
            outs = _bass_exec_p.bind(
                *operands,
                out_avals=tuple(out_avals),
                in_names=tuple(bound_names),
                out_names=tuple(self.out_names),
                lowering_input_output_aliases=(),
                sim_require_finite=True,
                sim_require_nnan=True,
                nc=nc,
            )
            return tuple(outs)

        import os
        if os.environ.get("BASS_CPU_SIM") == "1":
            devices = jax.devices("cpu")[:n_cores]
        else:
            devices = jax.devices()[:n_cores]
        assert len(devices) == n_cores, f"need {n_cores} cores, have {len(devices)}"
        self.mesh = Mesh(np.asarray(devices), ("core",))
        self.spec = PartitionSpec("core")
        self.sharding = NamedSharding(self.mesh, self.spec)
        n_args = n_params + len(out_names)
        self._jit = jax.jit(
            shard_map(
                _body,
                mesh=self.mesh,
                in_specs=(self.spec,) * n_args,
                out_specs=(self.spec,) * len(out_names),
                check_rep=False,
            ),
            keep_unused=True,
        )

    def put(self, in_maps):
        args = []
        for name in self.in_names:
            arrs = [np.asarray(in_maps[c][name]) for c in range(self.n_cores)]
            args.append(np.concatenate(arrs, axis=0))
        for z in self.zero_outs:
            args.append(np.concatenate([z] * self.n_cores, axis=0))
        return [self.jax.device_put(a, self.sharding) for a in args]

    def run_device(self, device_args):
        return self._jit(*device_args)

    def run(self, in_maps):
        outs = self._jit(*self.put(in_maps))
        np_outs = [np.asarray(o) for o in outs]
        results = []
        for c in range(self.n_cores):
            d = {}
            for i, name in enumerate(self.out_names):
                full = np_outs[i]
                per = full.shape[0] // self.n_cores
                d[name] = full[c * per:(c + 1) * per]
            results.append(d)
        return results


def _get_runner():
    global _RUNNER
    if _RUNNER is None:
        _RUNNER = _SpmdRunner(_build_nc(), N_CORES)
    return _RUNNER


def _prep_in_maps(hidden_states, user_mask, dense_W, dense_b, out_proj_W, out_proj_b):
    import ml_dtypes
    bf16 = ml_dtypes.bfloat16
    e4 = ml_dtypes.float8_e4m3

    x = np.ascontiguousarray(hidden_states[:, 0, :], dtype=np.float32)   # [B, H]

    def pack_x(a8):
        # [B, H] -> [p, t, cp, i, b]
        return np.ascontiguousarray(
            a8.reshape(NB, 128, NCP, 2, 128).transpose(4, 0, 2, 3, 1))

    xs = x * SX
    x_hi = xs.astype(e4)
    x_lo = (xs - x_hi.astype(np.float32)).astype(e4)
    x8_arr = np.ascontiguousarray(
        np.stack([pack_x(x_hi), pack_x(x_lo)], axis=2))  # [p, t, hl, cp, i, b]

    in_maps = []
    for c in range(N_CORES):
        sl = slice(c * KSL, (c + 1) * KSL)
        Ws = dense_W[:, :, sl] * SW                                      # [U, H, 96]
        w_hi = Ws.astype(e4)
        w_lo = (Ws - w_hi.astype(np.float32)).astype(e4)

        def pack_w(a8):
            # [u, H, KSL] -> [p, u, cp, i, k]
            nu = a8.shape[0]
            return a8.reshape(nu, NCP, 2, 128, KSL).transpose(3, 0, 1, 2, 4)

        w8s_arr = np.ascontiguousarray(pack_w(w_hi[:NSINGLE]))
        w8_arr = np.ascontiguousarray(
            np.stack([pack_w(w_hi[NSINGLE:]), pack_w(w_lo[NSINGLE:])],
                     axis=2))                            # [p, u, hl, cp, i, k]

        wolk = out_proj_W[:, sl, :].transpose(0, 2, 1).reshape(U, WOC)   # [U, (l,k)]
        md_arr = np.ascontiguousarray(np.concatenate(
            [user_mask.T, dense_b[:, sl], wolk, out_proj_b / N_CORES],
            axis=1)).astype(bf16)                                        # [U, 546]
        in_maps.append({
            "x8": x8_arr,
            "w8s": w8s_arr,
            "w8": w8_arr,
            "md": md_arr,
        })
    return in_maps


def kernel(hidden_states, user_mask, dense_W, dense_b, out_proj_W, out_proj_b):
    hidden_states = np.asarray(hidden_states, dtype=np.float32)
    user_mask = np.asarray(user_mask, dtype=np.float32)
    dense_W = np.asarray(dense_W, dtype=np.float32)
    dense_b = np.asarray(dense_b, dtype=np.float32)
    out_proj_W = np.asarray(out_proj_W, dtype=np.float32)
    out_proj_b = np.asarray(out_proj_b, dtype=np.float32)

    runner = _get_runner()
    in_maps = _prep_in_maps(hidden_states, user_mask, dense_W, dense_b,
                            out_proj_W, out_proj_b)
    results = runner.run(in_maps)
    out = np.zeros((B, L), np.float32)
    for c in range(N_CORES):
        oc = results[c]["o"][:, :8]              # [128, 8] (rest is padding)
        for t in range(NB):
            out[t * 128:(t + 1) * 128, :] += (
                oc[:, t * L:(t + 1) * L] + oc[:, 4 + t * L:4 + t * L + L])
    return out


# revision 3
# speedup vs baseline: 1.0043x; 1.0043x over previous
"""Trainium2 Bass kernel for nn_LongformerPersonalizedClsHead (MoE routing head).

Reference computation (B=256, S=512, H=768, U=100, L=2):
    x  = hidden_states[:, 0, :]                      # [B, H]  (CLS token only)
    z  = sum_u mask[b,u] * (x @ dense_W[u]) + mask @ dense_b
    h  = tanh(z)
    out= sum_u mask[b,u] * (h @ out_proj_W[u]) + mask @ out_proj_b   # [B, L]

Sharding: each core owns 96 k-columns of layer 1 (dense) output; the expert
sum is local per core, and layer 2 contracts over k so each core emits an
independent [256, 2+2] partial that the host sums. Zero on-device collectives.

fp8 hi/lo residual scheme: W and x are split on the host into
fp8e4m3 hi + fp8e4m3 lo (lo = quantization residual stored at the SAME scale,
so all passes share one PSUM accumulation group). Per (expert, batch-tile) the
PE runs 9 DoubleRow matmuls (contraction 256 each, 20 ns apiece at full
p-state) covering x_hi*w_hi + x_hi*w_lo + x_lo*w_hi; the dropped lo*lo term
is ~0.1% relative. With all experts dual (hi+lo) the end-to-end rel err is
3.1e-3 (bf16 baseline was 4.3e-3); the first NSINGLE=18 experts ship hi-only
(one less moving tensor + 6 instead of 9 matmuls), trading up to 1.44e-2
measured rel err (threshold 2e-2) for 18*205ns less DMA stream. PE drops from
48us (bf16) to ~33us, under the ~38.7us DMA stream, so the kernel is
DMA-stream-bound: 1966ns head + 38.7us stream + 900ns DMA-sem + last-chunk
matmuls + combine/tanh/reduce tail + writeback = 47.5us (vs 59.5us bf16).

Combine chains: mask-weighted accumulation of each expert's PSUM tile into z
runs on ACT(mul)+Pool(add) for batch-tile 0 and fused DVE stt for tile 1,
with TWO ping-pong accumulators per tile (a serial z+= chain self-paces at
~484ns/step, slower than the ~426ns/expert DMA pace; alternating experts
between two accumulators halves each chain's step rate).

Tail: Wo_comb[b,(l,k)] = sum_u mask[b,u]*out_proj_W[u,k,l] and the output
bias are precomputed early on the PE (one 194-wide matmul per batch tile from
maskT/woT that arrive in the DMA head), so the tail is just
tanh -> 2 fused multiply-reduce ops per tile -> one small writeback. No
transpose, no layer-2 matmul, no hT copy on the critical path.
"""
import numpy as np

B, S, H, U, L = 256, 512, 768, 100, 2
N_CORES = 8
KSL = H // N_CORES    # 96 k-columns per core
NB = B // 128         # 2 batch tiles
NCP = H // 256        # 3 contraction chunk-pairs (DoubleRow contracts 256)
WOC = 2 * KSL         # 192 = (l, k) columns of the combined layer-2 weight

# DMA chunk sizes for the expert-weight stream (experts per DMA). 2-expert
# chunks (819ns transfer) keep HWDGE (625ns/DMA) off the critical path. The
# tail shrinks GRADUALLY (PE time of chunk n, 360ns/expert, must stay under
# the transfer time of chunk n+1, 410ns/expert, or PE piles up behind the
# per-chunk +900ns DMA-sem latency) so the final expert's matmuls start
# 900ns after the last arrival with no backlog.
NSINGLE = 18          # experts stored hi-only (no w_lo pass): rel err ~1.4e-2,
                      # saves 205ns of stream per expert
NDUAL = U - NSINGLE
# single-expert chunks (hi-only, 205ns/expert transfer)
WS_CHUNKS = [6, 6, 6]
assert sum(WS_CHUNKS) == NSINGLE
# dual-expert chunks; the taper keeps PE(chunk n) under transfer(chunk n+1)
W_CHUNKS = [2] + [6] * 7 + [5, 5, 4, 4, 3, 3, 3, 2, 2, 2, 2, 2, 1]
assert sum(W_CHUNKS) == NDUAL
TAILK = 2             # last experts run t0-block then t1-block so the t0
                      # merge/tanh (Pool/ACT) overlap the t1 block
TRIGGER_OUT = False   # SWDGE prepare/trigger writeback; broken on real HW (see notes)
OPAD = 64             # o rows padded to 256B for the SWDGE scatter-add writeback

PE_WARMUP = 45        # dummy matmuls to ramp the PE clock during the DMA head
SX, SW = 16.0, 512.0  # fp8 pre-scales for x and W (power of 2, exact)

_RUNNER = None


def _build_nc():
    import concourse.bacc as bacc
    import concourse.mybir as mybir
    import concourse.tile as tile
    from concourse.masks import make_identity

    f32 = mybir.dt.float32
    bf16 = mybir.dt.bfloat16
    fp8 = mybir.dt.float8e4
    DR = mybir.MatmulPerfMode.DoubleRow
    mult = mybir.AluOpType.mult
    add = mybir.AluOpType.add

    nc = bacc.Bacc("TRN2", target_bir_lowering=False)

    # Host-prepacked layouts (p = partition = h % 128 within chunk-pair):
    # x8[p, t, hl, cp, i, b]: h = cp*256 + i*128 + p, b_global = t*128 + b
    x8 = nc.dram_tensor("x8", [128, NB, 2, NCP, 2, 128], fp8, kind="ExternalInput")
    # singles (experts 0..NSINGLE-1), hi only: w8s[p, u, cp, i, k]
    w8s = nc.dram_tensor("w8s", [128, NSINGLE, NCP, 2, KSL], fp8,
                         kind="ExternalInput")
    # duals (experts NSINGLE..U-1): w8[p, u, hl, cp, i, k], hl 0=hi 1=lo
    w8 = nc.dram_tensor("w8", [128, NDUAL, 2, NCP, 2, KSL], fp8,
                        kind="ExternalInput")
    # [maskT(256) | dense_b slice(96) | woT (l,k)(192) | out_proj_b.T/8 (2)]
    md = nc.dram_tensor("md", [U, B + KSL + WOC + L], bf16, kind="ExternalInput")
    # output (rows padded to 256B for the scatter-add writeback):
    # cols [t*L + l] = sum_k h*Wo_comb partials, cols [4 + t*L + l]
    # = (mask @ out_proj_b)/8 bias seeds; host sums cols 0..7.
    o = nc.dram_tensor("o", [128, OPAD], f32, kind="ExternalOutput")

    with tile.TileContext(nc) as tc:
        with (
            tc.tile_pool(name="const", bufs=1) as cpool,
            tc.tile_pool(name="tmp", bufs=6) as tpool,
            tc.tile_pool(name="py", bufs=7, space="PSUM") as psum_y,
            tc.tile_pool(name="pq", bufs=1, space="PSUM") as psum_q,
        ):
            # ones is the first DVE instruction of the program so the PE
            # warmup below can begin as early as possible (the 3us clock ramp
            # must complete before the first expert matmul to pay off)
            ones_first = cpool.tile([1, 128], bf16, tag="ones1")
            nc.vector.memset(ones_first[:], 1.0)

            # --- input DMAs ---
            # SP/HWDGE queue carries the critical stream (x8 halves + expert
            # weights); small inputs ride the Pool/SWDGE queue right behind
            # the first weight chunk.
            x8_sb = cpool.tile([128, NB, 2, NCP, 2, 128], fp8, tag="x8")
            nc.sync.dma_start(x8_sb[:, 0], x8[:, 0])
            ws_sb = cpool.tile([128, NSINGLE, NCP, 2, KSL], fp8, tag="w8s")
            nc.sync.dma_start(ws_sb[:, 0:WS_CHUNKS[0]], w8s[:, 0:WS_CHUNKS[0]])
            nc.sync.dma_start(x8_sb[:, 1], x8[:, 1])
            u0 = WS_CHUNKS[0]
            for n in WS_CHUNKS[1:]:
                nc.sync.dma_start(ws_sb[:, u0:u0 + n], w8s[:, u0:u0 + n])
                u0 += n
            w8_sb = cpool.tile([128, NDUAL, 2, NCP, 2, KSL], fp8, tag="w8")
            u0 = 0
            for n in W_CHUNKS:
                nc.sync.dma_start(w8_sb[:, u0:u0 + n], w8[:, u0:u0 + n])
                u0 += n

            md_sb = cpool.tile([U, B + KSL + WOC + L], bf16, tag="md")
            nc.gpsimd.dma_start(md_sb[:], md[:])
            # chain-combine scalars mask/(SX*SW) are reconstructed on-chip
            # from md's bf16 maskT (PE transpose + ACT scale-copy) instead of
            # paying a separate 284ns DMA in the critical stream
            mask_sb = cpool.tile([128, NB, U], f32, tag="mask")
            maskT_sb = md_sb[:, 0:B]
            db_sb = md_sb[:, B:B + KSL]
            wob_sb = md_sb[:, B + KSL:B + KSL + WOC + L]   # [woT | bo/8]

            o_pad = cpool.tile([128, OPAD], f32, tag="opad")
            nc.vector.memset(o_pad[:], 0.0)

            # SWDGE scatter-add writeback: descriptors prepared up-front on
            # the (idle-early) Pool engine; the end-of-kernel trigger then
            # pays only transfer + semaphore instead of HWDGE + DGE ~1.3us.
            # Token i reads its index from [p=i%16, s=i//16]; the interp views
            # the AP as [128, 8] and bounds-checks every value, so zero the
            # unread partitions >= 16.
            if TRIGGER_OUT:
                # scatter-add needs a known-zero destination; the output DRAM
                # buffer is not guaranteed zero on hardware. The zero-fill DMA
                # rides the SP queue after the last weight chunk (o_pad is
                # still all-zero here; its stt writers get a WAR edge on this
                # read, satisfied ~35us before they run).
                nc.sync.dma_start(o[:], o_pad[:])
                oidx = cpool.tile([128, 128 // 16], mybir.dt.int16, tag="oidx")
                nc.gpsimd.memset(oidx[:], 0)
                nc.gpsimd.iota(oidx[0:16, :], pattern=[[16, 128 // 16]], base=0,
                               channel_multiplier=1)
                dma_sem = nc.alloc_semaphore("swdge_out")
                nc.gpsimd.dma_scatter_add(
                    o[:], o_pad[:].rearrange("p (a e) -> p a e", a=1), oidx[:],
                    128, 128, OPAD, prepare_only=True, sem=dma_sem,
                )

            # z accumulators: [t][pp] with pp the ping-pong slot. Slot 0 is
            # seeded with the dense bias (PE matmul below); slot 1 starts 0.
            z_sb = cpool.tile([128, NB, 2, KSL], f32, tag="z")
            for t in range(NB):
                nc.vector.memset(z_sb[:, t, 1, :], 0.0)
            h_sb = cpool.tile([128, NB, KSL], bf16, tag="h")
            woc_sb = cpool.tile([128, NB, WOC], bf16, tag="woc")

            # p-state warmup: keep PE continuously busy through its ~3us clock
            # ramp while the head DMAs stream
            for _ in range(PE_WARMUP):
                wacc = psum_y.tile([128, KSL], f32, tag="y")
                nc.tensor.matmul(wacc[:], ones_first[:], ones_first[:, 0:KSL],
                                 start=True, stop=True)

            def emit_expert(u, t, dve_combine=False):
                acc = psum_y.tile([128, KSL], f32, tag="y")
                if u < NSINGLE:
                    # hi-only expert: z_u = (x_hi + x_lo) * w_hi
                    steps = [(0, cp) for cp in range(NCP)] + \
                            [(1, cp) for cp in range(NCP)]
                    for n, (xhl, cp) in enumerate(steps):
                        nc.tensor.matmul(
                            acc[:], x8_sb[:, t, xhl, cp], ws_sb[:, u, cp],
                            start=(n == 0), stop=(n == len(steps) - 1),
                            perf_mode=DR,
                        )
                else:
                    steps = [(0, 0, cp) for cp in range(NCP)] + \
                            [(0, 1, cp) for cp in range(NCP)] + \
                            [(1, 0, cp) for cp in range(NCP)]
                    for n, (xhl, whl, cp) in enumerate(steps):
                        nc.tensor.matmul(
                            acc[:],
                            x8_sb[:, t, xhl, cp],
                            w8_sb[:, u - NSINGLE, whl, cp],
                            start=(n == 0), stop=(n == len(steps) - 1),
                            perf_mode=DR,
                        )
                pp = (u + 1) % 2
                if t == 0 and not dve_combine:
                    tmp = tpool.tile([128, KSL], f32, tag="tmp")
                    nc.scalar.mul(tmp[:], acc[:], mask_sb[:, 0, u:u + 1])
                    nc.gpsimd.tensor_add(z_sb[:, 0, pp, :], z_sb[:, 0, pp, :],
                                         tmp[:])
                else:
                    nc.vector.scalar_tensor_tensor(
                        z_sb[:, t, pp, :], acc[:], mask_sb[:, t, u:u + 1],
                        z_sb[:, t, pp, :], op0=mult, op1=add,
                    )

            # --- seeds (PE): mask transpose, z bias, Wo_comb + out bias ---
            # These stall PE ~0.5us on the SWDGE-delivered md tensor, which is
            # free (PE has ~5us of slack against the weight stream). The mask
            # copies MUST precede the first combine mul in the ACT queue.
            ident = cpool.tile([128, 128], bf16, tag="ident")
            make_identity(nc, ident[:])
            for t in range(NB):
                mt = psum_y.tile([128, U], bf16, tag="y")
                nc.tensor.transpose(mt[:], maskT_sb[:, t * 128:(t + 1) * 128],
                                    ident[0:U, 0:U])
                nc.scalar.mul(mask_sb[:, t, :], mt[:], 1.0 / (SX * SW))
            for t in range(NB):
                ps = psum_y.tile([128, KSL], f32, tag="y")
                nc.tensor.matmul(
                    ps[:], maskT_sb[:, t * 128:(t + 1) * 128], db_sb[:],
                    start=True, stop=True,
                )
                nc.scalar.copy(z_sb[:, t, 0, :], ps[:])
            for t in range(NB):
                qp = psum_q.tile([128, WOC + L], f32, tag="q")
                nc.tensor.matmul(
                    qp[:], maskT_sb[:, t * 128:(t + 1) * 128], wob_sb[:],
                    start=True, stop=True,
                )
                nc.vector.tensor_copy(woc_sb[:, t, :], qp[:, 0:WOC])
                nc.scalar.copy(o_pad[:, 4 + t * L:4 + t * L + L],
                               qp[:, WOC:WOC + L])

            emit_expert(0, 0)
            emit_expert(0, 1)
            emit_expert(1, 0)
            emit_expert(1, 1)
            for u in range(2, U - TAILK):
                emit_expert(u, 0)
                emit_expert(u, 1)

            # --- tail ---
            # Last TAILK experts run all-t0 then all-t1; the t0 combines jump
            # to the 1-op DVE stt path so the t0 chain drains without the
            # ACT->Pool 2-hop latency. merge0 on Pool, merge1 on DVE, the two
            # tanh on ACT, and all four multiply-reduce ops on DVE (walrus
            # rejects TensorScalarPtr on Pool).
            def stt_reduce(t, l):
                p2 = tpool.tile([128, KSL], f32, tag="p2")
                nc.vector.scalar_tensor_tensor(
                    p2[:], h_sb[:, t, :], 1.0,
                    woc_sb[:, t, l * KSL:(l + 1) * KSL],
                    op0=mult, op1=mult,
                    accum_out=o_pad[:, t * L + l:t * L + l + 1],
                )

            for u in range(U - TAILK, U):
                emit_expert(u, 0, dve_combine=True)
            nc.gpsimd.tensor_add(z_sb[:, 0, 0, :], z_sb[:, 0, 0, :],
                                 z_sb[:, 0, 1, :])
            nc.scalar.activation(h_sb[:, 0, :], z_sb[:, 0, 0, :],
                                 mybir.ActivationFunctionType.Tanh)
            for u in range(U - TAILK, U):
                emit_expert(u, 1)
            nc.vector.tensor_add(z_sb[:, 1, 0, :], z_sb[:, 1, 0, :],
                                 z_sb[:, 1, 1, :])
            nc.scalar.activation(h_sb[:, 1, :], z_sb[:, 1, 0, :],
                                 mybir.ActivationFunctionType.Tanh)
            for l in range(L):
                stt_reduce(0, l)
            for l in range(L):
                stt_reduce(1, l)
            if TRIGGER_OUT:
                nc.gpsimd.trigger_dma(count=None)
            else:
                nc.sync.dma_start(o[:, 0:8], o_pad[:, 0:8])

    # Tile schedules data consumers of the prepared scatter-add against the
    # prep's DMASW lane sem, but the completion increment it bakes stays on
    # the user-provided sem (on_update[0]), which the trigger-drain fires in
    # both CoreSim and TimelineSim. Rewire every wait on the orphaned DMASW
    # sem to wait on the user sem instead; the DMASW sem then stays untouched
    # (cleared at zero) and the teardown's sync check is satisfied.
    fn = nc.m.functions[0]
    if not TRIGGER_OUT:
        nc.finalize()
        return nc
    upd_ids = set()
    user_sem = None
    for blk in fn.blocks:
        for inst in blk.instructions:
            si = inst.sync_info
            if not si:
                continue
            if "ScatterAdd" in type(inst).__name__:
                user_sem = si.on_update[0]
                assert user_sem.ant_name == "swdge_out", user_sem
            for upd in si.on_update:
                if (upd.ant_name or "").startswith("DMASW"):
                    upd_ids.add(upd.id)
    assert user_sem is not None
    for blk in fn.blocks:
        for inst in blk.instructions:
            si = inst.sync_info
            if not si:
                continue
            for wt in si.on_wait:
                if (wt.ant_name or "").startswith("DMASW") and wt.id not in upd_ids:
                    wt.id, wt.ant_name = user_sem.id, user_sem.ant_name

    nc.finalize()
    return nc


class _SpmdRunner:
    """Cached PJRT SPMD runner (mirrors concourse.bass2jax.run_bass_via_pjrt,
    but keeps the jitted callable alive so repeat calls don't re-trace)."""

    def __init__(self, nc, n_cores):
        import jax
        import concourse.mybir as mybir
        from concourse.bass2jax import (
            _bass_exec_p, install_neuronx_cc_hook, partition_id_tensor,
        )
        from jax.sharding import Mesh, PartitionSpec, NamedSharding
        try:
            from jax.experimental.shard_map import shard_map
        except ImportError:
            from jax.shard_map import shard_map

        install_neuronx_cc_hook()
        self.jax = jax
        self.nc = nc
        self.n_cores = n_cores

        in_names, out_names, out_avals, zero_outs = [], [], [], []
        partition_name = nc.partition_id_tensor.name if nc.partition_id_tensor else None
        dbg_name = None
        if nc.dbg_addr is not None:
            assert not nc.dbg_callbacks
            dbg_name = nc.dbg_addr.name
        for alloc in nc.m.functions[0].allocations:
            if not isinstance(alloc, mybir.MemoryLocationSet):
                continue
            name = alloc.memorylocations[0].name
            if alloc.kind == "ExternalInput":
                if name not in (partition_name, dbg_name):
                    in_names.append(name)
            elif alloc.kind == "ExternalOutput":
                out_names.append(name)
                shape = tuple(alloc.tensor_shape)
                dtype = mybir.dt.np(alloc.dtype)
                out_avals.append(jax.core.ShapedArray(shape, dtype))
                zero_outs.append(np.zeros(shape, dtype))

        self.in_names = list(in_names)
        self.out_names = list(out_names)
        self.zero_outs = zero_outs

        n_params = len(in_names)
        bound_names = list(in_names) + list(out_names)
        if dbg_name is not None:
            bound_names.append(dbg_name)
        if partition_name is not None:
            bound_names.append(partition_name)

        def _body(*args):
            operands = list(args)
            if dbg_name is not None:
                operands.append(jax.numpy.zeros((1, 2), jax.numpy.uint32))
            if partition_name is not None:
                operands.append(partition_id_tensor())
            outs = _bass_exec_p.bind(
                *operands,
                out_avals=tuple(out_avals),
                in_names=tuple(bound_names),
                out_names=tuple(self.out_names),
                lowering_input_output_aliases=(),
                sim_require_finite=True,
                sim_require_nnan=True,
                nc=nc,
            )
            return tuple(outs)

        import os
        if os.environ.get("BASS_CPU_SIM") == "1":
            devices = jax.devices("cpu")[:n_cores]
        else:
            devices = jax.devices()[:n_cores]
        assert len(devices) == n_cores, f"need {n_cores} cores, have {len(devices)}"
        self.mesh = Mesh(np.asarray(devices), ("core",))
        self.spec = PartitionSpec("core")
        self.sharding = NamedSharding(self.mesh, self.spec)
        n_args = n_params + len(out_names)
        self._jit = jax.jit(
            shard_map(
                _body,
                mesh=self.mesh,
                in_specs=(self.spec,) * n_args,
                out_specs=(self.spec,) * len(out_names),
                check_rep=False,
            ),
            keep_unused=True,
        )

    def put(self, in_maps):
        args = []
        for name in self.in_names:
            arrs = [np.asarray(in_maps[c][name]) for c in range(self.n_cores)]
            args.append(np.concatenate(arrs, axis=0))
        for z in self.zero_outs:
            args.append(np.concatenate([z] * self.n_cores, axis=0))
        return [self.jax.device_put(a, self.sharding) for a in args]

    def run_device(self, device_args):
        return self._jit(*device_args)

    def run(self, in_maps):
        outs = self._jit(*self.put(in_maps))
        np_outs = [np.asarray(o) for o in outs]
        results = []
        for c in range(self.n_cores):
            d = {}
            for i, name in enumerate(self.out_names):
                full = np_outs[i]
                per = full.shape[0] // self.n_cores
                d[name] = full[c * per:(c + 1) * per]
            results.append(d)
        return results


def _get_runner():
    global _RUNNER
    if _RUNNER is None:
        _RUNNER = _SpmdRunner(_build_nc(), N_CORES)
    return _RUNNER


def _prep_in_maps(hidden_states, user_mask, dense_W, dense_b, out_proj_W, out_proj_b):
    import ml_dtypes
    bf16 = ml_dtypes.bfloat16
    e4 = ml_dtypes.float8_e4m3

    x = np.ascontiguousarray(hidden_states[:, 0, :], dtype=np.float32)   # [B, H]

    def pack_x(a8):
        # [B, H] -> [p, t, cp, i, b]
        return np.ascontiguousarray(
            a8.reshape(NB, 128, NCP, 2, 128).transpose(4, 0, 2, 3, 1))

    xs = x * SX
    x_hi = xs.astype(e4)
    x_lo = (xs - x_hi.astype(np.float32)).astype(e4)
    x8_arr = np.ascontiguousarray(
        np.stack([pack_x(x_hi), pack_x(x_lo)], axis=2))  # [p, t, hl, cp, i, b]

    in_maps = []
    for c in range(N_CORES):
        sl = slice(c * KSL, (c + 1) * KSL)
        Ws = dense_W[:, :, sl] * SW                                      # [U, H, 96]
        w_hi = Ws.astype(e4)
        w_lo = (Ws - w_hi.astype(np.float32)).astype(e4)

        def pack_w(a8):
            # [u, H, KSL] -> [p, u, cp, i, k]
            nu = a8.shape[0]
            return a8.reshape(nu, NCP, 2, 128, KSL).transpose(3, 0, 1, 2, 4)

        w8s_arr = np.ascontiguousarray(pack_w(w_hi[:NSINGLE]))
        w8_arr = np.ascontiguousarray(
            np.stack([pack_w(w_hi[NSINGLE:]), pack_w(w_lo[NSINGLE:])],
                     axis=2))                            # [p, u, hl, cp, i, k]

        wolk = out_proj_W[:, sl, :].transpose(0, 2, 1).reshape(U, WOC)   # [U, (l,k)]
        md_arr = np.ascontiguousarray(np.concatenate(
            [user_mask.T, dense_b[:, sl], wolk, out_proj_b / N_CORES],
            axis=1)).astype(bf16)                                        # [U, 546]
        in_maps.append({
            "x8": x8_arr,
            "w8s": w8s_arr,
            "w8": w8_arr,
            "md": md_arr,
        })
    return in_maps


def kernel(hidden_states, user_mask, dense_W, dense_b, out_proj_W, out_proj_b):
    hidden_states = np.asarray(hidden_states, dtype=np.float32)
    user_mask = np.asarray(user_mask, dtype=np.float32)
    dense_W = np.asarray(dense_W, dtype=np.float32)
    dense_b = np.asarray(dense_b, dtype=np.float32)
    out_proj_W = np.asarray(out_proj_W, dtype=np.float32)
    out_proj_b = np.asarray(out_proj_b, dtype=np.float32)

    runner = _get_runner()
    in_maps = _prep_in_maps(hidden_states, user_mask, dense_W, dense_b,
                            out_proj_W, out_proj_b)
    results = runner.run(in_maps)
    out = np.zeros((B, L), np.float32)
    for c in range(N_CORES):
        oc = results[c]["o"][:, :8]              # [128, 8] (rest is padding)
        for t in range(NB):
            out[t * 128:(t + 1) * 128, :] += (
                oc[:, t * L:(t + 1) * L] + oc[:, 4 + t * L:4 + t * L + L])
    return out
